# revision 1
# baseline (speedup 1.0000x reference)
import numpy as np
import jax
import jax.numpy as jnp

# Problem constants (nn_AdaTTSp): hardcoded per harness rules.
L, T, E, D, H = 2, 8, 2, 128, 128
NE = T * E  # 16
M = 8  # number of NeuronCores; data-parallel over batch

_BF = jnp.bfloat16
_F32 = jnp.float32


def _forward(x, w1, b1, w2, b2, gate_w, gate_b, sewf):
    # x: [b, T, D] local shard. Weights pre-cast to bf16 on host; biases f32.
    # sewf: [L, T, NE] — self-expert residual pre-scattered into gate space.
    for l in range(L):
        xb = x.astype(_BF)
        # Expert MLP: w1[l] reshaped [T, E, D, H] so no repeat() is needed.
        h = jax.nn.relu(
            jnp.einsum('btd,tedh->bteh', xb, w1[l],
                       preferred_element_type=_F32) + b1[l])
        eo = jax.nn.relu(
            jnp.einsum('bteh,teho->bteo', h.astype(_BF), w2[l],
                       preferred_element_type=_F32) + b2[l])  # [b,T,E,H]
        eo = eo.reshape(eo.shape[0], NE, H)
        # Gating over all NE experts per task; fold self-expert residual in.
        logits = jnp.einsum('btd,tde->bte', xb, gate_w[l],
                            preferred_element_type=_F32) + gate_b[l]
        coef = jax.nn.softmax(logits, axis=-1) + sewf[l]  # [b, T, NE]
        x = jnp.einsum('bte,beh->bth', coef.astype(_BF), eo.astype(_BF),
                       preferred_element_type=_F32)
    return x


_pfwd = jax.pmap(_forward, axis_name='x',
                 in_axes=(0, None, None, None, None, None, None, None))


def _prep(w1, b1, w2, b2, gate_w, gate_b, sew):
    # Host-side weight prep (tiny tensors): layouts + bf16 cast + sew scatter.
    w1r = np.asarray(w1, np.float32).reshape(L, T, E, D, H)
    b1r = np.asarray(b1, np.float32).reshape(L, T, E, H)
    w2r = np.asarray(w2, np.float32).reshape(L, T, E, H, H)
    b2r = np.asarray(b2, np.float32).reshape(L, T, E, H)
    sewf = np.zeros((L, T, NE), np.float32)
    for t in range(T):
        for e in range(E):
            sewf[:, t, t * E + e] = np.asarray(sew)[:, t, e]
    bf = jnp.bfloat16
    return (jnp.asarray(w1r, bf), jnp.asarray(b1r), jnp.asarray(w2r, bf),
            jnp.asarray(b2r), jnp.asarray(np.asarray(gate_w, np.float32), bf),
            jnp.asarray(np.asarray(gate_b, np.float32)), jnp.asarray(sewf))


def kernel(inputs, w1, b1, w2, b2, gate_w, gate_b, sew):
    B = inputs.shape[0]
    xs = np.asarray(inputs).reshape(M, B // M, T, D)
    wargs = _prep(w1, b1, w2, b2, gate_w, gate_b, sew)
    out = _pfwd(xs, *wargs)
    return np.asarray(out).reshape(B, T, H).astype(np.float32)



# revision 3
# speedup vs baseline: 200.3110x; 200.3110x over previous
"""Bass/Tile kernel for nn_AdaTTSp (2-layer multi-task MoE, AdaTT-style).

Math per layer l (reference):
    x: [B, T=8, D=128]
    h  = relu(einsum('bed,edh->beh', repeat(x,E=2,axis=1), w1[l]) + b1[l])
    eo = relu(einsum('beh,eho->beo', h, w2[l]) + b2[l])            # [B, NE=16, H]
    g  = softmax(einsum('btd,tde->bte', x, gate_w[l]) + gate_b[l]) # [B, T, NE]
    x' = einsum('bte,beh->bth', g + scatter(sew[l]), eo)

Strategy: data-parallel over batch across 8 NeuronCores (4096 rows/core).
Per core, per 512-row block, per layer:
  - DMA-transpose loads xT_t [D, 512] per task (fp16)
  - MM1 (PE): hT_te = relu(w1.T @ xT + b1) with per-partition bias via ACT
  - gates (PE block-diag logits accumulation + ones-matmul bias, ACT exp,
    DVE segmented reduce + reciprocal, coef = exp/Z + sew via STT)
  - MM2 (PE): eo_te row-major = relu(hT.T @ w2 + ones@b2) per 128-row chunk
  - mixing (DVE): fused_t = sum_j coef[b,t,j] * eo_j[b,:] via
    scalar_tensor_tensor multiply-accumulate chains (fp16)
Layer 0 writes fp16 scratch in DRAM; layer 1 reads it back via DMA-transpose
and writes the f32 output.
"""
import numpy as np
from concourse import bacc, bass, tile
from concourse.bass_utils import run_bass_kernel_spmd

mybir = bass.mybir
FP16 = mybir.dt.float16
F32 = mybir.dt.float32
AF = mybir.ActivationFunctionType
OP = mybir.AluOpType

L, T, E, D, H, B = 2, 8, 2, 128, 128, 32768
NE = T * E           # 16 experts per layer
M = 8                # cores
BLOC = B // M        # 4096 rows per core
BLK = 512            # rows per block (matmul moving free dim)
CH = 128             # rows per chunk (psum partition dim)

_nc_cache = {}


def build_nc(bloc=BLOC):
    if bloc in _nc_cache:
        return _nc_cache[bloc]
    nblk = bloc // BLK
    nch = BLK // CH

    nc = bacc.Bacc(None, target_bir_lowering=False)
    x = nc.dram_tensor("x", [bloc, T, D], FP16, kind="ExternalInput")
    w1d = nc.dram_tensor("w1d", [D, L, NE, H], FP16, kind="ExternalInput")
    w2d = nc.dram_tensor("w2d", [H, L, NE, H], FP16, kind="ExternalInput")
    b1d = nc.dram_tensor("b1d", [H, L, NE], F32, kind="ExternalInput")
    b2d = nc.dram_tensor("b2d", [1, L, NE, H], FP16, kind="ExternalInput")
    gwd = nc.dram_tensor("gwd", [D, L, T, NE * T], FP16, kind="ExternalInput")
    gbd = nc.dram_tensor("gbd", [1, L, NE * T], FP16, kind="ExternalInput")
    sewd = nc.dram_tensor("sewd", [128, L, T, NE], F32, kind="ExternalInput")
    y = nc.dram_tensor("y", [bloc, T, H], F32, kind="ExternalOutput")

    with tile.TileContext(nc) as tc:
        with (
            tc.tile_pool(name="wpool", bufs=1) as wpool,
            tc.tile_pool(name="dram", bufs=1, space="DRAM") as dpool,
            tc.tile_pool(name="xpool", bufs=2) as xpool,
            tc.tile_pool(name="hpool", bufs=2) as hpool,
            tc.tile_pool(name="epool", bufs=3) as epool,
            tc.tile_pool(name="cpool", bufs=4) as cpool,
            tc.tile_pool(name="mpool", bufs=4) as mpool,
            tc.tile_pool(name="opool", bufs=3) as opool,
            tc.tile_pool(name="ph", bufs=2, space="PSUM") as phpool,
            tc.tile_pool(name="pg", bufs=2, space="PSUM") as pgpool,
            tc.tile_pool(name="pe", bufs=3, space="PSUM") as pepool,
        ):
            # persistent weights in SBUF
            w1t = wpool.tile([D, L, NE, H], FP16)
            w2t = wpool.tile([H, L, NE, H], FP16)
            b1t = wpool.tile([H, L, NE], F32)
            b2t = wpool.tile([1, L, NE, H], FP16)
            gwt = wpool.tile([D, L, T, NE * T], FP16)
            gbt = wpool.tile([1, L, NE * T], FP16)
            sewt = wpool.tile([128, L, T, NE], F32)
            onest = wpool.tile([1, CH], FP16)
            for tl, dr in ((w1t, w1d), (w2t, w2d), (b1t, b1d), (b2t, b2d),
                           (gwt, gwd), (gbt, gbd), (sewt, sewd)):
                nc.sync.dma_start(tl[:], dr[:])
            nc.vector.memset(onest[:], 1.0)

            fused0 = dpool.tile([bloc, T, H], FP16)

            for li in range(L):
                src = x if li == 0 else fused0
                for blk in range(nblk):
                    r0 = blk * BLK
                    # transposed activation loads: xT [D, T, BLK]
                    xT = xpool.tile([D, T, BLK], FP16)
                    for t in range(T):
                        nc.sync.dma_start_transpose(
                            xT[:, t, :], src[r0:r0 + BLK, t, :])
                    # MM1 + relu/bias eviction: hT [H, NE, BLK]
                    hT = hpool.tile([H, NE, BLK], FP16)
                    for te in range(NE):
                        ph = phpool.tile([H, BLK], F32)
                        nc.tensor.matmul(ph[:], w1t[:, li, te, :],
                                         xT[:, te // E, :],
                                         start=True, stop=True)
                        nc.scalar.activation(hT[:, te, :], ph[:], AF.Relu,
                                             bias=b1t[:, li, te:te + 1])
                    for c in range(nch):
                        c0 = c * CH
                        rows = r0 + c0
                        # --- gates for this 128-row chunk ---
                        pg = pgpool.tile([CH, NE * T], F32)
                        for t in range(T):
                            nc.tensor.matmul(pg[:], xT[:, t, c0:c0 + CH],
                                             gwt[:, li, t, :],
                                             start=(t == 0), stop=False)
                        nc.tensor.matmul(pg[:], onest[:], gbt[:, li, :],
                                         start=False, stop=True)
                        expt = cpool.tile([CH, T, NE], F32)
                        nc.scalar.activation(expt[:], pg[:], AF.Exp)
                        rsum = cpool.tile([CH, T], F32)
                        nc.vector.tensor_reduce(rsum[:], expt[:],
                                                axis=mybir.AxisListType.X,
                                                op=OP.add)
                        rcp = cpool.tile([CH, T], F32)
                        nc.vector.reciprocal(rcp[:], rsum[:])
                        coef = cpool.tile([CH, T, NE], F32)
                        for t in range(T):
                            nc.vector.scalar_tensor_tensor(
                                coef[:, t, :], expt[:, t, :], rcp[:, t:t + 1],
                                sewt[:, li, t, :], op0=OP.mult, op1=OP.add)
                        # --- expert layer 2 (row-major eo) ---
                        eo = epool.tile([CH, NE, H], FP16)
                        for te in range(NE):
                            pe = pepool.tile([CH, H], F32)
                            nc.tensor.matmul(pe[:], hT[:, te, c0:c0 + CH],
                                             w2t[:, li, te, :],
                                             start=True, stop=False)
                            nc.tensor.matmul(pe[:], onest[:],
                                             b2t[:, li, te, :],
                                             start=False, stop=True)
                            nc.scalar.activation(eo[:, te, :], pe[:], AF.Relu)
                        # --- mixing on DVE ---
                        ot = opool.tile([CH, T, H], FP16)
                        for t in range(T):
                            acc = mpool.tile([CH, H], FP16)
                            nc.vector.tensor_scalar_mul(
                                acc[:], eo[:, 0, :], coef[:, t, 0:1])
                            for j in range(1, NE):
                                dst = acc[:] if j < NE - 1 else ot[:, t, :]
                                nc.vector.scalar_tensor_tensor(
                                    dst, eo[:, j, :], coef[:, t, j:j + 1],
                                    acc[:], op0=OP.mult, op1=OP.add)
                        if li == 0:
                            nc.sync.dma_start(
                                fused0[rows:rows + CH, :, :], ot[:])
                        else:
                            yt = opool.tile([CH, T, H], F32)
                            nc.scalar.activation(yt[:], ot[:], AF.Copy)
                            nc.sync.dma_start(y[rows:rows + CH, :, :], yt[:])
    nc.compile()
    _nc_cache[bloc] = nc
    return nc


def prep_weights(w1, b1, w2, b2, gate_w, gate_b, sew):
    """Host-side packing of the (tiny) weight tensors into kernel layouts."""
    w1 = np.asarray(w1, np.float32)
    w2 = np.asarray(w2, np.float32)
    b1 = np.asarray(b1, np.float32)
    b2 = np.asarray(b2, np.float32)
    gate_w = np.asarray(gate_w, np.float32)
    gate_b = np.asarray(gate_b, np.float32)
    sew = np.asarray(sew, np.float32)

    w1d = np.ascontiguousarray(w1.transpose(2, 0, 1, 3)).astype(np.float16)
    w2d = np.ascontiguousarray(w2.transpose(2, 0, 1, 3)).astype(np.float16)
    b1d = np.ascontiguousarray(b1.transpose(2, 0, 1)).astype(np.float32)
    b2d = b2.reshape(1, L, NE, H).astype(np.float16)
    # block-diagonal padded gate weights: col t*NE+j of task t's logits
    gwp = np.zeros((L, T, D, NE * T), np.float32)
    for l in range(L):
        for t in range(T):
            gwp[l, t, :, t * NE:(t + 1) * NE] = gate_w[l, t]
    gwd = np.ascontiguousarray(gwp.transpose(2, 0, 1, 3)).astype(np.float16)
    gbd = gate_b.reshape(1, L, NE * T).astype(np.float16)
    # self-expert residual scattered into gate space, partition-broadcast
    sewf = np.zeros((L, T, NE), np.float32)
    for t in range(T):
        for e in range(E):
            sewf[:, t, t * E + e] = sew[:, t, e]
    sewd = np.broadcast_to(sewf[None], (128, L, T, NE)).copy()
    return {"w1d": w1d, "w2d": w2d, "b1d": b1d, "b2d": b2d,
            "gwd": gwd, "gbd": gbd, "sewd": sewd}


def kernel(inputs, w1, b1, w2, b2, gate_w, gate_b, sew):
    nc = build_nc()
    wmap = prep_weights(w1, b1, w2, b2, gate_w, gate_b, sew)
    xs = np.asarray(inputs, np.float16).reshape(M, BLOC, T, D)
    in_maps = [{"x": xs[i], **wmap} for i in range(M)]
    r = run_bass_kernel_spmd(nc, in_maps, core_ids=list(range(M)))
    out = np.concatenate([r.results[i]["y"] for i in range(M)], axis=0)
    return out.astype(np.float32, copy=False)


# revision 7
# speedup vs baseline: 8002.8184x; 39.9520x over previous
"""Bass/Tile kernel for nn_AdaTTSp (2-layer multi-task MoE, AdaTT-style).

Math per layer l (reference):
    x: [B, T=8, D=128]
    h  = relu(einsum('bed,edh->beh', repeat(x,E=2,axis=1), w1[l]) + b1[l])
    eo = relu(einsum('beh,eho->beo', h, w2[l]) + b2[l])            # [B, NE=16, H]
    g  = softmax(einsum('btd,tde->bte', x, gate_w[l]) + gate_b[l]) # [B, T, NE]
    x' = einsum('bte,beh->bth', g + scatter(sew[l]), eo)

Strategy: data-parallel over batch across 8 NeuronCores (4096 rows/core).
Per core, per 512-row block, per layer:
  - DMA-transpose loads xT_t [D, 512] per task (fp16)
  - MM1 (PE): hT_te = relu(w1.T @ xT + b1) with per-partition bias via ACT
  - gates (PE block-diag logits accumulation + ones-matmul bias, ACT exp,
    DVE segmented reduce + reciprocal, coef = exp/Z + sew via STT)
  - MM2 (PE): eo_te row-major = relu(hT.T @ w2 + ones@b2) per 128-row chunk
  - mixing (DVE): fused_t = sum_j coef[b,t,j] * eo_j[b,:] via
    scalar_tensor_tensor multiply-accumulate chains (fp16)
Layer 0 writes fp16 scratch in DRAM; layer 1 reads it back via DMA-transpose
and writes the f32 output.
"""
import numpy as np
from concourse import bacc, bass, tile
from concourse.bass_utils import run_bass_kernel_spmd

mybir = bass.mybir
FP16 = mybir.dt.float16
F32 = mybir.dt.float32
AF = mybir.ActivationFunctionType
OP = mybir.AluOpType

L, T, E, D, H, B = 2, 8, 2, 128, 128, 32768
NE = T * E           # 16 experts per layer
M = 8                # cores
BLOC = B // M        # 4096 rows per core
BLK = 512            # rows per block (matmul moving free dim)
CH = 128             # rows per chunk (psum partition dim)

_nc_cache = {}


def build_nc(bloc=BLOC, reps=1, skip=()):
    """skip: subset of {'mix','mm2','gates','mm1','loads','stores'} for
    ablation timing (produces wrong results)."""
    key = (bloc, reps, tuple(sorted(skip)))
    if key in _nc_cache:
        return _nc_cache[key]
    nblk = bloc // BLK
    nch = BLK // CH

    nc = bacc.Bacc(None, target_bir_lowering=False)
    x = nc.dram_tensor("x", [bloc, T, D], FP16, kind="ExternalInput")
    w1d = nc.dram_tensor("w1d", [D, L, NE, H], FP16, kind="ExternalInput")
    w2d = nc.dram_tensor("w2d", [H, L, NE, H], FP16, kind="ExternalInput")
    b1d = nc.dram_tensor("b1d", [H, L, NE], F32, kind="ExternalInput")
    b2d = nc.dram_tensor("b2d", [1, L, NE, H], FP16, kind="ExternalInput")
    gwd = nc.dram_tensor("gwd", [D, L, T, NE * T], FP16, kind="ExternalInput")
    gbd = nc.dram_tensor("gbd", [1, L, NE * T], FP16, kind="ExternalInput")
    sewd = nc.dram_tensor("sewd", [128, L, T, NE], F32, kind="ExternalInput")
    y = nc.dram_tensor("y", [bloc, T, H], F32, kind="ExternalOutput")

    with tile.TileContext(nc) as tc:
        with (
            tc.tile_pool(name="wpool", bufs=1) as wpool,
            tc.tile_pool(name="dram", bufs=1, space="DRAM") as dpool,
            tc.tile_pool(name="xpool", bufs=2) as xpool,
            tc.tile_pool(name="hpool", bufs=2) as hpool,
            tc.tile_pool(name="epool", bufs=3) as epool,
            tc.tile_pool(name="cpool", bufs=4) as cpool,
            tc.tile_pool(name="mpool", bufs=4) as mpool,
            tc.tile_pool(name="opool", bufs=3) as opool,
            tc.tile_pool(name="ph", bufs=2, space="PSUM") as phpool,
            tc.tile_pool(name="pg", bufs=2, space="PSUM") as pgpool,
            tc.tile_pool(name="pe", bufs=3, space="PSUM") as pepool,
        ):
            # persistent weights in SBUF
            w1t = wpool.tile([D, L, NE, H], FP16)
            w2t = wpool.tile([H, L, NE, H], FP16)
            b1t = wpool.tile([H, L, NE], F32)
            b2t = wpool.tile([1, L, NE, H], FP16)
            gwt = wpool.tile([D, L, T, NE * T], FP16)
            gbt = wpool.tile([1, L, NE * T], FP16)
            sewt = wpool.tile([128, L, T, NE], F32)
            onest = wpool.tile([1, CH], FP16)
            for tl, dr in ((w1t, w1d), (w2t, w2d), (b1t, b1d), (b2t, b2d),
                           (gwt, gwd), (gbt, gbd), (sewt, sewd)):
                nc.sync.dma_start(tl[:], dr[:])
            nc.vector.memset(onest[:], 1.0)

            fused0 = dpool.tile([bloc, T, H], FP16)

            for _rep in range(reps):
              for li in range(L):
                src = x if li == 0 else fused0
                for blk in range(nblk):
                    r0 = blk * BLK
                    # transposed activation loads: xT [D, T, BLK]
                    xT = xpool.tile([D, T, BLK], FP16)
                    if 'loads' not in skip:
                        for t in range(T):
                            nc.sync.dma_start_transpose(
                                xT[:, t, :], src[r0:r0 + BLK, t, :])
                    # MM1 + relu/bias eviction: hT [H, NE, BLK]
                    hT = hpool.tile([H, NE, BLK], FP16)
                    if 'mm1' not in skip:
                        for te in range(NE):
                            ph = phpool.tile([H, BLK], F32)
                            nc.tensor.matmul(ph[:], w1t[:, li, te, :],
                                             xT[:, te // E, :],
                                             start=True, stop=True)
                            nc.scalar.activation(hT[:, te, :], ph[:], AF.Relu,
                                                 bias=b1t[:, li, te:te + 1])
                    for c in range(nch):
                        c0 = c * CH
                        rows = r0 + c0
                        # --- gates for this 128-row chunk ---
                        coef = cpool.tile([CH, T, NE], F32)
                        if 'gates' not in skip:
                            pg = pgpool.tile([CH, NE * T], F32)
                            for t in range(T):
                                nc.tensor.matmul(pg[:], xT[:, t, c0:c0 + CH],
                                                 gwt[:, li, t, :],
                                                 start=(t == 0), stop=False)
                            nc.tensor.matmul(pg[:], onest[:], gbt[:, li, :],
                                             start=False, stop=True)
                            expt = cpool.tile([CH, T, NE], F32)
                            nc.scalar.activation(expt[:], pg[:], AF.Exp)
                            rsum = cpool.tile([CH, T], F32)
                            nc.vector.tensor_reduce(rsum[:], expt[:],
                                                    axis=mybir.AxisListType.X,
                                                    op=OP.add)
                            rcp = cpool.tile([CH, T], F32)
                            nc.vector.reciprocal(rcp[:], rsum[:])
                            for t in range(T):
                                nc.vector.scalar_tensor_tensor(
                                    coef[:, t, :], expt[:, t, :],
                                    rcp[:, t:t + 1], sewt[:, li, t, :],
                                    op0=OP.mult, op1=OP.add)
                        # --- expert layer 2 (row-major eo) ---
                        eo = epool.tile([CH, NE, H], FP16)
                        if 'mm2' not in skip:
                            for te in range(NE):
                                pe = pepool.tile([CH, H], F32)
                                nc.tensor.matmul(pe[:], hT[:, te, c0:c0 + CH],
                                                 w2t[:, li, te, :],
                                                 start=True, stop=False)
                                nc.tensor.matmul(pe[:], onest[:],
                                                 b2t[:, li, te, :],
                                                 start=False, stop=True)
                                nc.scalar.activation(eo[:, te, :], pe[:],
                                                     AF.Relu)
                        # --- mixing on DVE ---
                        # j-outer, t-inner: the 8 per-task accumulation
                        # chains interleave so consecutive DVE ops are
                        # independent (chained RAW costs 411ns/op vs 172ns).
                        ot = opool.tile([CH, T, H], FP16)
                        if 'mix' not in skip:
                            accs = [mpool.tile([CH, H], FP16, name=f"acc{t}",
                                               tag=f"acc{t}")
                                    for t in range(T)]
                            for t in range(T):
                                nc.vector.tensor_scalar_mul(
                                    accs[t][:], eo[:, 0, :], coef[:, t, 0:1])
                            for j in range(1, NE):
                                for t in range(T):
                                    dst = (accs[t][:] if j < NE - 1
                                           else ot[:, t, :])
                                    nc.vector.scalar_tensor_tensor(
                                        dst, eo[:, j, :], coef[:, t, j:j + 1],
                                        accs[t][:], op0=OP.mult, op1=OP.add)
                        if 'stores' not in skip:
                            if li == 0:
                                nc.sync.dma_start(
                                    fused0[rows:rows + CH, :, :], ot[:])
                            else:
                                yt = opool.tile([CH, T, H], F32)
                                nc.scalar.activation(yt[:], ot[:], AF.Copy)
                                nc.sync.dma_start(
                                    y[rows:rows + CH, :, :], yt[:])
    nc.compile()
    _nc_cache[key] = nc
    return nc


def prep_weights(w1, b1, w2, b2, gate_w, gate_b, sew):
    """Host-side packing of the (tiny) weight tensors into kernel layouts."""
    w1 = np.asarray(w1, np.float32)
    w2 = np.asarray(w2, np.float32)
    b1 = np.asarray(b1, np.float32)
    b2 = np.asarray(b2, np.float32)
    gate_w = np.asarray(gate_w, np.float32)
    gate_b = np.asarray(gate_b, np.float32)
    sew = np.asarray(sew, np.float32)

    w1d = np.ascontiguousarray(w1.transpose(2, 0, 1, 3)).astype(np.float16)
    w2d = np.ascontiguousarray(w2.transpose(2, 0, 1, 3)).astype(np.float16)
    b1d = np.ascontiguousarray(b1.transpose(2, 0, 1)).astype(np.float32)
    b2d = b2.reshape(1, L, NE, H).astype(np.float16)
    # block-diagonal padded gate weights: col t*NE+j of task t's logits
    gwp = np.zeros((L, T, D, NE * T), np.float32)
    for l in range(L):
        for t in range(T):
            gwp[l, t, :, t * NE:(t + 1) * NE] = gate_w[l, t]
    gwd = np.ascontiguousarray(gwp.transpose(2, 0, 1, 3)).astype(np.float16)
    gbd = gate_b.reshape(1, L, NE * T).astype(np.float16)
    # self-expert residual scattered into gate space, partition-broadcast
    sewf = np.zeros((L, T, NE), np.float32)
    for t in range(T):
        for e in range(E):
            sewf[:, t, t * E + e] = sew[:, t, e]
    sewd = np.broadcast_to(sewf[None], (128, L, T, NE)).copy()
    return {"w1d": w1d, "w2d": w2d, "b1d": b1d, "b2d": b2d,
            "gwd": gwd, "gbd": gbd, "sewd": sewd}


def kernel(inputs, w1, b1, w2, b2, gate_w, gate_b, sew):
    nc = build_nc()
    wmap = prep_weights(w1, b1, w2, b2, gate_w, gate_b, sew)
    xs = np.asarray(inputs, np.float16).reshape(M, BLOC, T, D)
    in_maps = [{"x": xs[i], **wmap} for i in range(M)]
    r = run_bass_kernel_spmd(nc, in_maps, core_ids=list(range(M)))
    out = np.concatenate([r.results[i]["y"] for i in range(M)], axis=0)
    return out.astype(np.float32, copy=False)
